# revision 9
# baseline (speedup 1.0000x reference)
"""DeepAR (2-layer LSTM, B=1024, W=288, H=128) forward on 8 Trainium2 cores.

Pure data-parallel: batch 1024 -> 128 per core; weights replicated.

Device layout is "transposed activations": every on-chip tensor is
(feature_dim = partitions, batch = free).  The LSTM cell uses the identity
sigmoid(x) = (tanh(x/2)+1)/2 so that ONE tanh activation op covers all four
gates; the i/f/o gate rows of all weights are pre-halved on the host to
compensate.  Cell state is stored as C = 2c and hidden as H = 2h (weights
consuming h are pre-halved), which lets every elementwise step be a single
fused scalar_tensor_tensor op:

    u     = (Ti + 1) * Tg
    v     = (Tf + 1) * C_prev
    C_new = 0.5*v + u                  (== 2*c_new)
    tanhc = tanh(0.5 * C_new)          (ACT with scale=0.5)
    H_new = (To + 1) * tanhc           (== 2*h_new)

Prediction-phase feedback (prev_y = mean_{t-1}) is folded into the recurrence
as a rank-1 matrix Wfb = Wi0[:,0] (x) (0.5*meanW) applied to H2, so the mean
head never runs inside the loop; means are computed on the host from the
exported H2 states.
"""

import numpy as np

B = 1024
SEQ, PRED = 192, 96
W = SEQ + PRED  # 288
HID = 128
NCORES = 8
BS = B // NCORES  # 128
IN = 67
KX = IN + 1  # + indicator row carrying pred-phase feedback bias
G4 = 4 * HID  # 512
# torch gate order (i, f, g, o) -> device order (i, f, o, g)
GATE_PERM = [0, 1, 3, 2]
X_CHUNK = 16  # scan steps per input-DMA chunk
# packed weight-constant column offsets
WOFF = {"wi0": 0, "wh0": 512, "wi1": 1024, "wh1": 1536, "wfb": 2048,
        "b1m": 2560, "b2m": 2688, "bones": 2816}
WCOLS = 2816 + 512  # 3328


def _perm_rows(w):
    """(4H, X) or (4H,) -> gate-permuted + i/f/o rows halved (tanh trick)."""
    w = w.reshape(4, HID, -1) if w.ndim == 2 else w.reshape(4, HID, 1)
    w = w[GATE_PERM].astype(np.float64).copy()
    w[0] *= 0.5  # i
    w[1] *= 0.5  # f
    w[2] *= 0.5  # o
    return w  # (4, HID, X)


def _as_blocksT(w4):
    """(4, HID, K) -> (K, 4*HID) with gate blocks along columns (lhsT form)."""
    k = w4.shape[2]
    out = np.zeros((k, G4), np.float64)
    for g in range(4):
        out[:, g * HID:(g + 1) * HID] = w4[g].T
    return out


def host_prep(inputs):
    """All data-movement-only preprocessing + weight folding. Returns dict."""
    f32 = np.float32
    ge = np.asarray(inputs["given_enc"], f32)
    x_enc = np.asarray(inputs["x_enc"], f32)
    xm = np.asarray(inputs["x_mark_enc"], f32)
    mx = np.asarray(inputs["meta_x"], f32)
    tembs = [np.asarray(inputs[f"time_emb{i}"], f32) for i in range(3)]
    membs = [np.asarray(inputs[f"meta_emb{i}"], f32) for i in range(2)]

    tcat = ge[:, :, 4:7].astype(np.int32)
    time_feat = np.concatenate(
        [ge[:, :, :4]] + [tembs[i][tcat[:, :, i]] for i in range(3)], axis=-1
    )  # (B, W, 28)
    mcat = mx[:, 2:4].astype(np.int32)
    meta_feat = np.concatenate(
        [mx[:, :2]] + [membs[i][mcat[:, i]] for i in range(2)], axis=-1
    )  # (B, 34)

    nm = x_enc.mean(axis=1, keepdims=True)  # (B,1,1)
    xc = x_enc - nm
    ns = np.sqrt(xc.var(axis=1, keepdims=True) + 1e-5)
    xn = (xc / ns).astype(f32)  # (B, SEQ, 1)

    teacher = np.zeros((B, W, 1), f32)
    teacher[:, 0] = xn[:, 0]
    teacher[:, 1:SEQ] = xn[:, : SEQ - 1]
    ind = np.zeros((B, W, 1), f32)
    ind[:, SEQ:] = 1.0
    xfeat = np.concatenate(
        [teacher, time_feat, xm,
         np.broadcast_to(meta_feat[:, None, :], (B, W, 34)), ind],
        axis=-1,
    )  # (B, W, 68)

    Wi0 = np.asarray(inputs["W_ih0"], np.float64)  # (512, 67)
    Wh0 = np.asarray(inputs["W_hh0"], np.float64)
    Wi1 = np.asarray(inputs["W_ih1"], np.float64)
    Wh1 = np.asarray(inputs["W_hh1"], np.float64)
    b1 = np.asarray(inputs["b_ih0"], np.float64) + np.asarray(inputs["b_hh0"], np.float64)
    b2 = np.asarray(inputs["b_ih1"], np.float64) + np.asarray(inputs["b_hh1"], np.float64)
    meanW = np.asarray(inputs["mean_W"], np.float64)  # (1, 128)
    mean_b = float(np.asarray(inputs["mean_b"]).reshape(()))

    meanW_h = 0.5 * meanW  # applied to H2 = 2*h2
    wfb_full = Wi0[:, 0:1] @ meanW_h  # (512, 128), consumes H2
    bias_fb = Wi0[:, 0] * mean_b  # (512,)

    wi0T = _as_blocksT(_perm_rows(Wi0))  # (67, 512)
    wi0T_aug = np.zeros((KX, G4), np.float64)
    wi0T_aug[:IN] = wi0T
    wi0T_aug[IN] = _as_blocksT(_perm_rows(bias_fb)).reshape(G4)  # indicator row
    wh0T = _as_blocksT(_perm_rows(Wh0) * 0.5)  # (128, 512); *0.5 for H=2h
    wi1T = _as_blocksT(_perm_rows(Wi1) * 0.5)
    wh1T = _as_blocksT(_perm_rows(Wh1) * 0.5)
    wfbT = _as_blocksT(_perm_rows(wfb_full))  # (128, 512)

    b1m = _perm_rows(b1).reshape(4, HID)  # (4, 128)
    b2m = _perm_rows(b2).reshape(4, HID)
    bones = np.zeros((4, G4), f32)
    for g in range(4):
        bones[g, g * HID:(g + 1) * HID] = 1.0

    # per-core transposed inputs: (KX, W*BS), feature on partitions
    xt_cores = []
    for c in range(NCORES):
        xf = xfeat[c * BS:(c + 1) * BS]  # (BS, W, KX)
        xt = np.ascontiguousarray(xf.transpose(2, 1, 0)).reshape(KX, W * BS)
        xt_cores.append(xt.astype(f32))

    # Pack every weight into one (128, WCOLS) tensor -> single DMA -> single
    # semaphore for all weight-consuming matmuls (walrus allows only one sync
    # wait per LDWEIGHTS).
    wconst = np.zeros((HID, WCOLS), f32)
    wconst[:KX, WOFF["wi0"]:WOFF["wi0"] + G4] = wi0T_aug
    wconst[:, WOFF["wh0"]:WOFF["wh0"] + G4] = wh0T
    wconst[:, WOFF["wi1"]:WOFF["wi1"] + G4] = wi1T
    wconst[:, WOFF["wh1"]:WOFF["wh1"] + G4] = wh1T
    wconst[:, WOFF["wfb"]:WOFF["wfb"] + G4] = wfbT
    wconst[:4, WOFF["b1m"]:WOFF["b1m"] + HID] = b1m
    wconst[:4, WOFF["b2m"]:WOFF["b2m"] + HID] = b2m
    wconst[:4, WOFF["bones"]:WOFF["bones"] + G4] = bones

    return dict(
        xt_cores=xt_cores,
        wconst=wconst,
        weights=dict(
            wi0=wi0T_aug.astype(f32), wh0=wh0T.astype(f32),
            wi1=wi1T.astype(f32), wh1=wh1T.astype(f32),
            wfb=wfbT.astype(f32), b1m=b1m.astype(f32),
            b2m=b2m.astype(f32), bones=bones,
        ),
        meanW_h=meanW_h.astype(f32), mean_b=mean_b,
        norm_std=ns.astype(f32), norm_mean=nm.astype(f32),
    )


def host_post(h2_cores, prep):
    """h2_cores: list of (PRED, HID, BS) arrays of H2=2*h2. -> (B, PRED, 1)."""
    meanW_h = prep["meanW_h"][0]  # (HID,)
    out = np.empty((B, PRED, 1), np.float32)
    for c, h2 in enumerate(h2_cores):
        # mean_norm[t, b] = sum_h meanW_h[h] * H2[t, h, b] + mean_b
        mn = np.einsum("h,thb->bt", meanW_h, h2.astype(np.float32)) + prep["mean_b"]
        out[c * BS:(c + 1) * BS, :, 0] = mn
    out = out * prep["norm_std"] + prep["norm_mean"]
    return out.astype(np.float32)


def build_bass():
    import concourse.bass as bass  # noqa: F401
    import concourse.tile as tile
    from concourse import bacc, mybir

    f32 = mybir.dt.float32
    AF = mybir.ActivationFunctionType
    ALU = mybir.AluOpType

    nc = bacc.Bacc("TRN2", target_bir_lowering=False, num_devices=NCORES)
    xt_d = nc.dram_tensor("xt", [KX, W * BS], f32, kind="ExternalInput")
    wc_d = nc.dram_tensor("wconst", [HID, WCOLS], f32, kind="ExternalInput")
    h2out_d = nc.dram_tensor("h2out", [PRED, HID, BS], f32, kind="ExternalOutput")

    with tile.TileContext(nc) as tc:
        with (
            tc.tile_pool(name="const", bufs=1) as const,
            tc.tile_pool(name="xin", bufs=3) as xin,
            tc.tile_pool(name="state", bufs=3) as state,
            tc.tile_pool(name="work", bufs=3) as work,
            tc.tile_pool(name="ps", bufs=2, space="PSUM") as ps,
        ):
            wc = const.tile([HID, WCOLS], f32, tag="wc", name="wc")
            nc.sync.dma_start(out=wc, in_=wc_d[:, :])
            wt = {
                "wi0": wc[:KX, WOFF["wi0"]:WOFF["wi0"] + G4],
                "wh0": wc[:, WOFF["wh0"]:WOFF["wh0"] + G4],
                "wi1": wc[:, WOFF["wi1"]:WOFF["wi1"] + G4],
                "wh1": wc[:, WOFF["wh1"]:WOFF["wh1"] + G4],
                "wfb": wc[:, WOFF["wfb"]:WOFF["wfb"] + G4],
                "b1m": wc[:4, WOFF["b1m"]:WOFF["b1m"] + HID],
                "b2m": wc[:4, WOFF["b2m"]:WOFF["b2m"] + HID],
                "bones": wc[:4, WOFF["bones"]:WOFF["bones"] + G4],
            }

            def new_state(tag):
                t = state.tile([HID, BS], f32, tag=tag, name=tag)
                nc.vector.memset(t, 0.0)
                return t

            h1, c1 = new_state("h1"), new_state("c1")
            h2, c2 = new_state("h2"), new_state("c2")

            def blk(ap, g):
                return ap[:, g * HID:(g + 1) * HID]

            def cell(gates_ps, cprev, tag):
                """tanh + fused elementwise; returns (h_new, c_new)."""
                t = work.tile([HID, G4], f32, tag=f"t{tag}", name=f"t{tag}")
                nc.scalar.activation(out=t, in_=gates_ps, func=AF.Tanh)
                ti, tf, to, tg = (blk(t, g) for g in range(4))
                u = work.tile([HID, BS], f32, tag=f"u{tag}", name=f"u{tag}")
                nc.vector.scalar_tensor_tensor(
                    out=u, in0=ti, scalar=1.0, in1=tg, op0=ALU.add, op1=ALU.mult)
                v = work.tile([HID, BS], f32, tag=f"v{tag}", name=f"v{tag}")
                nc.vector.scalar_tensor_tensor(
                    out=v, in0=tf, scalar=1.0, in1=cprev, op0=ALU.add, op1=ALU.mult)
                c_new = state.tile([HID, BS], f32, tag=f"c{tag}", name=f"c{tag}")
                nc.vector.scalar_tensor_tensor(
                    out=c_new, in0=v, scalar=0.5, in1=u, op0=ALU.mult, op1=ALU.add)
                tc_ = work.tile([HID, BS], f32, tag=f"tc{tag}", name=f"tc{tag}")
                nc.scalar.activation(out=tc_, in_=c_new, func=AF.Tanh, scale=0.5)
                h_new = state.tile([HID, BS], f32, tag=f"h{tag}", name=f"h{tag}")
                nc.vector.scalar_tensor_tensor(
                    out=h_new, in0=to, scalar=1.0, in1=tc_, op0=ALU.add, op1=ALU.mult)
                return h_new, c_new

            xt_sb = None
            for t in range(W):
                if t % X_CHUNK == 0:
                    nsteps = min(X_CHUNK, W - t)
                    xt_sb = xin.tile([KX, X_CHUNK * BS], f32, tag="xt", name="xt_sb")
                    nc.sync.dma_start(
                        out=xt_sb[:, : nsteps * BS],
                        in_=xt_d[:, t * BS:(t + nsteps) * BS])
                xcol = xt_sb[:, (t % X_CHUNK) * BS:(t % X_CHUNK + 1) * BS]

                g1 = ps.tile([HID, G4], f32, tag="g1", name="g1")
                nc.tensor.matmul(g1, lhsT=wt["b1m"], rhs=wt["bones"],
                                 start=True, stop=False)
                pred = t >= SEQ
                for g in range(4):
                    nc.tensor.matmul(blk(g1, g), lhsT=blk(wt["wi0"], g),
                                     rhs=xcol, start=False, stop=False)
                    last = (g == 3) and not pred
                    nc.tensor.matmul(blk(g1, g), lhsT=blk(wt["wh0"], g),
                                     rhs=h1, start=False, stop=last)
                    if pred:
                        nc.tensor.matmul(blk(g1, g), lhsT=blk(wt["wfb"], g),
                                         rhs=h2, start=False, stop=(g == 3))
                h1, c1 = cell(g1, c1, "1")

                g2 = ps.tile([HID, G4], f32, tag="g2", name="g2")
                nc.tensor.matmul(g2, lhsT=wt["b2m"], rhs=wt["bones"],
                                 start=True, stop=False)
                for g in range(4):
                    nc.tensor.matmul(blk(g2, g), lhsT=blk(wt["wi1"], g),
                                     rhs=h1, start=False, stop=False)
                    nc.tensor.matmul(blk(g2, g), lhsT=blk(wt["wh1"], g),
                                     rhs=h2, start=False, stop=(g == 3))
                h2, c2 = cell(g2, c2, "2")

                if t >= SEQ:
                    nc.sync.dma_start(out=h2out_d[t - SEQ], in_=h2)
    nc.compile()
    return nc


_BASS_CACHE = {}


def _get_bass():
    if "nc" not in _BASS_CACHE:
        _BASS_CACHE["nc"] = build_bass()
    return _BASS_CACHE["nc"]


def run(inputs, trace=False):
    """Returns (output, BassKernelResults)."""
    from concourse.bass_utils import run_bass_kernel_spmd

    prep = host_prep(inputs)
    nc = _get_bass()
    in_maps = [{"xt": prep["xt_cores"][c], "wconst": prep["wconst"]}
               for c in range(NCORES)]
    res = run_bass_kernel_spmd(nc, in_maps, core_ids=list(range(NCORES)),
                               trace=trace)
    h2_cores = [r["h2out"] for r in res.results]
    return host_post(h2_cores, prep), res


def kernel(**inputs) -> np.ndarray:
    out, _ = run(inputs, trace=False)
    return out


# revision 10
# speedup vs baseline: 1.6845x; 1.6845x over previous
"""DeepAR (2-layer LSTM, B=1024, W=288, H=128) forward on 8 Trainium2 cores.

Pure data-parallel: batch 1024 -> 128 per core; weights replicated.

Device layout is "transposed activations": every on-chip tensor is
(feature_dim = partitions, batch = free).  The LSTM cell uses the identity
sigmoid(x) = (tanh(x/2)+1)/2 so that ONE tanh activation op covers all four
gates; the i/f/o gate rows of all weights are pre-halved on the host to
compensate.  Cell state is stored as C = 2c and hidden as H = 2h (weights
consuming h are pre-halved), which lets every elementwise step be a single
fused scalar_tensor_tensor op:

    u     = (Ti + 1) * Tg
    v     = (Tf + 1) * C_prev
    C_new = 0.5*v + u                  (== 2*c_new)
    tanhc = tanh(0.5 * C_new)          (ACT with scale=0.5)
    H_new = (To + 1) * tanhc           (== 2*h_new)

Prediction-phase feedback (prev_y = mean_{t-1}) is folded into the recurrence
as a rank-1 matrix Wfb = Wi0[:,0] (x) (0.5*meanW) applied to H2, so the mean
head never runs inside the loop; means are computed on the host from the
exported H2 states.
"""

import ml_dtypes
import numpy as np

BF16 = ml_dtypes.bfloat16

B = 1024
SEQ, PRED = 192, 96
W = SEQ + PRED  # 288
HID = 128
NCORES = 8
BS = B // NCORES  # 128
IN = 67
KX = IN + 1  # + indicator row carrying pred-phase feedback bias
G4 = 4 * HID  # 512
# torch gate order (i, f, g, o) -> device order (i, f, o, g)
GATE_PERM = [0, 1, 3, 2]
X_CHUNK = 16  # scan steps per input-DMA chunk
# packed weight-constant column offsets
WOFF = {"wi0": 0, "wh0": 512, "wi1": 1024, "wh1": 1536, "wfb": 2048,
        "b1m": 2560, "b2m": 2688, "bones": 2816}
WCOLS = 2816 + 512  # 3328


def _perm_rows(w):
    """(4H, X) or (4H,) -> gate-permuted + i/f/o rows halved (tanh trick)."""
    w = w.reshape(4, HID, -1) if w.ndim == 2 else w.reshape(4, HID, 1)
    w = w[GATE_PERM].astype(np.float64).copy()
    w[0] *= 0.5  # i
    w[1] *= 0.5  # f
    w[2] *= 0.5  # o
    return w  # (4, HID, X)


def _as_blocksT(w4):
    """(4, HID, K) -> (K, 4*HID) with gate blocks along columns (lhsT form)."""
    k = w4.shape[2]
    out = np.zeros((k, G4), np.float64)
    for g in range(4):
        out[:, g * HID:(g + 1) * HID] = w4[g].T
    return out


def host_prep(inputs):
    """All data-movement-only preprocessing + weight folding. Returns dict."""
    f32 = np.float32
    ge = np.asarray(inputs["given_enc"], f32)
    x_enc = np.asarray(inputs["x_enc"], f32)
    xm = np.asarray(inputs["x_mark_enc"], f32)
    mx = np.asarray(inputs["meta_x"], f32)
    tembs = [np.asarray(inputs[f"time_emb{i}"], f32) for i in range(3)]
    membs = [np.asarray(inputs[f"meta_emb{i}"], f32) for i in range(2)]

    tcat = ge[:, :, 4:7].astype(np.int32)
    time_feat = np.concatenate(
        [ge[:, :, :4]] + [tembs[i][tcat[:, :, i]] for i in range(3)], axis=-1
    )  # (B, W, 28)
    mcat = mx[:, 2:4].astype(np.int32)
    meta_feat = np.concatenate(
        [mx[:, :2]] + [membs[i][mcat[:, i]] for i in range(2)], axis=-1
    )  # (B, 34)

    nm = x_enc.mean(axis=1, keepdims=True)  # (B,1,1)
    xc = x_enc - nm
    ns = np.sqrt(xc.var(axis=1, keepdims=True) + 1e-5)
    xn = (xc / ns).astype(f32)  # (B, SEQ, 1)

    teacher = np.zeros((B, W, 1), f32)
    teacher[:, 0] = xn[:, 0]
    teacher[:, 1:SEQ] = xn[:, : SEQ - 1]
    ind = np.zeros((B, W, 1), f32)
    ind[:, SEQ:] = 1.0
    xfeat = np.concatenate(
        [teacher, time_feat, xm,
         np.broadcast_to(meta_feat[:, None, :], (B, W, 34)), ind],
        axis=-1,
    )  # (B, W, 68)

    Wi0 = np.asarray(inputs["W_ih0"], np.float64)  # (512, 67)
    Wh0 = np.asarray(inputs["W_hh0"], np.float64)
    Wi1 = np.asarray(inputs["W_ih1"], np.float64)
    Wh1 = np.asarray(inputs["W_hh1"], np.float64)
    b1 = np.asarray(inputs["b_ih0"], np.float64) + np.asarray(inputs["b_hh0"], np.float64)
    b2 = np.asarray(inputs["b_ih1"], np.float64) + np.asarray(inputs["b_hh1"], np.float64)
    meanW = np.asarray(inputs["mean_W"], np.float64)  # (1, 128)
    mean_b = float(np.asarray(inputs["mean_b"]).reshape(()))

    meanW_h = 0.5 * meanW  # applied to H2 = 2*h2
    wfb_full = Wi0[:, 0:1] @ meanW_h  # (512, 128), consumes H2
    bias_fb = Wi0[:, 0] * mean_b  # (512,)

    wi0T = _as_blocksT(_perm_rows(Wi0))  # (67, 512)
    wi0T_aug = np.zeros((KX, G4), np.float64)
    wi0T_aug[:IN] = wi0T
    wi0T_aug[IN] = _as_blocksT(_perm_rows(bias_fb)).reshape(G4)  # indicator row
    wh0T = _as_blocksT(_perm_rows(Wh0) * 0.5)  # (128, 512); *0.5 for H=2h
    wi1T = _as_blocksT(_perm_rows(Wi1) * 0.5)
    wh1T = _as_blocksT(_perm_rows(Wh1) * 0.5)
    wfbT = _as_blocksT(_perm_rows(wfb_full))  # (128, 512)

    b1m = _perm_rows(b1).reshape(4, HID)  # (4, 128)
    b2m = _perm_rows(b2).reshape(4, HID)
    bones = np.zeros((4, G4), f32)
    for g in range(4):
        bones[g, g * HID:(g + 1) * HID] = 1.0

    # per-core transposed inputs: (KX, W*BS), feature on partitions
    xt_cores = []
    for c in range(NCORES):
        xf = xfeat[c * BS:(c + 1) * BS]  # (BS, W, KX)
        xt = np.ascontiguousarray(xf.transpose(2, 1, 0)).reshape(KX, W * BS)
        xt_cores.append(xt.astype(BF16))

    # Pack every weight into one (128, WCOLS) tensor -> single DMA -> single
    # semaphore for all weight-consuming matmuls (walrus allows only one sync
    # wait per LDWEIGHTS).
    wconst = np.zeros((HID, WCOLS), BF16)
    wconst[:KX, WOFF["wi0"]:WOFF["wi0"] + G4] = wi0T_aug
    wconst[:, WOFF["wh0"]:WOFF["wh0"] + G4] = wh0T
    wconst[:, WOFF["wi1"]:WOFF["wi1"] + G4] = wi1T
    wconst[:, WOFF["wh1"]:WOFF["wh1"] + G4] = wh1T
    wconst[:, WOFF["wfb"]:WOFF["wfb"] + G4] = wfbT
    wconst[:4, WOFF["b1m"]:WOFF["b1m"] + HID] = b1m
    wconst[:4, WOFF["b2m"]:WOFF["b2m"] + HID] = b2m
    wconst[:4, WOFF["bones"]:WOFF["bones"] + G4] = bones

    return dict(
        xt_cores=xt_cores,
        wconst=wconst,
        weights=dict(
            wi0=wi0T_aug.astype(f32), wh0=wh0T.astype(f32),
            wi1=wi1T.astype(f32), wh1=wh1T.astype(f32),
            wfb=wfbT.astype(f32), b1m=b1m.astype(f32),
            b2m=b2m.astype(f32), bones=bones,
        ),
        meanW_h=meanW_h.astype(f32), mean_b=mean_b,
        norm_std=ns.astype(f32), norm_mean=nm.astype(f32),
    )


def host_post(h2_cores, prep):
    """h2_cores: list of (PRED, HID, BS) arrays of H2=2*h2. -> (B, PRED, 1)."""
    meanW_h = prep["meanW_h"][0]  # (HID,)
    out = np.empty((B, PRED, 1), np.float32)
    for c, h2 in enumerate(h2_cores):
        # mean_norm[t, b] = sum_h meanW_h[h] * H2[t, h, b] + mean_b
        mn = np.einsum("h,thb->bt", meanW_h, h2.astype(np.float32)) + prep["mean_b"]
        out[c * BS:(c + 1) * BS, :, 0] = mn
    out = out * prep["norm_std"] + prep["norm_mean"]
    return out.astype(np.float32)


def build_bass():
    import concourse.bass as bass  # noqa: F401
    import concourse.tile as tile
    from concourse import bacc, mybir

    f32 = mybir.dt.float32
    bf16 = mybir.dt.bfloat16
    AF = mybir.ActivationFunctionType
    ALU = mybir.AluOpType

    nc = bacc.Bacc("TRN2", target_bir_lowering=False, num_devices=NCORES)
    xt_d = nc.dram_tensor("xt", [KX, W * BS], bf16, kind="ExternalInput")
    wc_d = nc.dram_tensor("wconst", [HID, WCOLS], bf16, kind="ExternalInput")
    h2out_d = nc.dram_tensor("h2out", [PRED, HID, BS], bf16, kind="ExternalOutput")

    with tile.TileContext(nc) as tc:
        with (
            tc.tile_pool(name="const", bufs=1) as const,
            tc.tile_pool(name="xin", bufs=3) as xin,
            tc.tile_pool(name="state", bufs=3) as state,
            tc.tile_pool(name="work", bufs=3) as work,
            tc.tile_pool(name="ps", bufs=2, space="PSUM") as ps,
        ):
            wc = const.tile([HID, WCOLS], bf16, tag="wc", name="wc")
            nc.sync.dma_start(out=wc, in_=wc_d[:, :])
            wt = {
                "wi0": wc[:KX, WOFF["wi0"]:WOFF["wi0"] + G4],
                "wh0": wc[:, WOFF["wh0"]:WOFF["wh0"] + G4],
                "wi1": wc[:, WOFF["wi1"]:WOFF["wi1"] + G4],
                "wh1": wc[:, WOFF["wh1"]:WOFF["wh1"] + G4],
                "wfb": wc[:, WOFF["wfb"]:WOFF["wfb"] + G4],
                "b1m": wc[:4, WOFF["b1m"]:WOFF["b1m"] + HID],
                "b2m": wc[:4, WOFF["b2m"]:WOFF["b2m"] + HID],
                "bones": wc[:4, WOFF["bones"]:WOFF["bones"] + G4],
            }

            def new_state(tag, dt=f32):
                t = state.tile([HID, BS], dt, tag=tag, name=tag)
                nc.vector.memset(t, 0.0)
                return t

            h1, c1 = new_state("h1", bf16), new_state("c1")
            h2, c2 = new_state("h2", bf16), new_state("c2")

            def blk(ap, g):
                return ap[:, g * HID:(g + 1) * HID]

            def cell(gates_ps, cprev, tag):
                """tanh + fused elementwise; returns (h_new, c_new)."""
                t = work.tile([HID, G4], bf16, tag=f"t{tag}", name=f"t{tag}")
                nc.scalar.activation(out=t, in_=gates_ps, func=AF.Tanh)
                ti, tf, to, tg = (blk(t, g) for g in range(4))
                u = work.tile([HID, BS], bf16, tag=f"u{tag}", name=f"u{tag}")
                nc.vector.scalar_tensor_tensor(
                    out=u, in0=ti, scalar=1.0, in1=tg, op0=ALU.add, op1=ALU.mult)
                v = work.tile([HID, BS], f32, tag=f"v{tag}", name=f"v{tag}")
                nc.vector.scalar_tensor_tensor(
                    out=v, in0=tf, scalar=1.0, in1=cprev, op0=ALU.add, op1=ALU.mult)
                c_new = state.tile([HID, BS], f32, tag=f"c{tag}", name=f"c{tag}")
                nc.vector.scalar_tensor_tensor(
                    out=c_new, in0=v, scalar=0.5, in1=u, op0=ALU.mult, op1=ALU.add)
                tc_ = work.tile([HID, BS], bf16, tag=f"tc{tag}", name=f"tc{tag}")
                nc.scalar.activation(out=tc_, in_=c_new, func=AF.Tanh, scale=0.5)
                h_new = state.tile([HID, BS], bf16, tag=f"h{tag}", name=f"h{tag}")
                nc.vector.scalar_tensor_tensor(
                    out=h_new, in0=to, scalar=1.0, in1=tc_, op0=ALU.add, op1=ALU.mult)
                return h_new, c_new

            xt_sb = None
            for t in range(W):
                if t % X_CHUNK == 0:
                    nsteps = min(X_CHUNK, W - t)
                    xt_sb = xin.tile([KX, X_CHUNK * BS], bf16, tag="xt", name="xt_sb")
                    nc.sync.dma_start(
                        out=xt_sb[:, : nsteps * BS],
                        in_=xt_d[:, t * BS:(t + nsteps) * BS])
                xcol = xt_sb[:, (t % X_CHUNK) * BS:(t % X_CHUNK + 1) * BS]

                g1 = ps.tile([HID, G4], f32, tag="g1", name="g1")
                nc.tensor.matmul(g1, lhsT=wt["b1m"], rhs=wt["bones"],
                                 start=True, stop=False)
                pred = t >= SEQ
                for g in range(4):
                    nc.tensor.matmul(blk(g1, g), lhsT=blk(wt["wi0"], g),
                                     rhs=xcol, start=False, stop=False)
                    last = (g == 3) and not pred
                    nc.tensor.matmul(blk(g1, g), lhsT=blk(wt["wh0"], g),
                                     rhs=h1, start=False, stop=last)
                    if pred:
                        nc.tensor.matmul(blk(g1, g), lhsT=blk(wt["wfb"], g),
                                         rhs=h2, start=False, stop=(g == 3))
                h1, c1 = cell(g1, c1, "1")

                g2 = ps.tile([HID, G4], f32, tag="g2", name="g2")
                nc.tensor.matmul(g2, lhsT=wt["b2m"], rhs=wt["bones"],
                                 start=True, stop=False)
                for g in range(4):
                    nc.tensor.matmul(blk(g2, g), lhsT=blk(wt["wi1"], g),
                                     rhs=h1, start=False, stop=False)
                    nc.tensor.matmul(blk(g2, g), lhsT=blk(wt["wh1"], g),
                                     rhs=h2, start=False, stop=(g == 3))
                h2, c2 = cell(g2, c2, "2")

                if t >= SEQ:
                    nc.sync.dma_start(out=h2out_d[t - SEQ], in_=h2)
    nc.compile()
    return nc


_BASS_CACHE = {}


def _get_bass():
    if "nc" not in _BASS_CACHE:
        _BASS_CACHE["nc"] = build_bass()
    return _BASS_CACHE["nc"]


def run(inputs, trace=False):
    """Returns (output, BassKernelResults)."""
    from concourse.bass_utils import run_bass_kernel_spmd

    prep = host_prep(inputs)
    nc = _get_bass()
    in_maps = [{"xt": prep["xt_cores"][c], "wconst": prep["wconst"]}
               for c in range(NCORES)]
    res = run_bass_kernel_spmd(nc, in_maps, core_ids=list(range(NCORES)),
                               trace=trace)
    h2_cores = [r["h2out"] for r in res.results]
    return host_post(h2_cores, prep), res


def kernel(**inputs) -> np.ndarray:
    out, _ = run(inputs, trace=False)
    return out


# revision 12
# speedup vs baseline: 1.9432x; 1.1536x over previous
"""DeepAR (2-layer LSTM, B=1024, W=288, H=128) forward on 8 Trainium2 cores.

Pure data-parallel: batch 1024 -> 128 per core; weights replicated.

Device layout is "transposed activations": every on-chip tensor is
(feature_dim = partitions, batch = free).  The LSTM cell uses the identity
sigmoid(x) = (tanh(x/2)+1)/2 so that ONE tanh activation op covers all four
gates; the i/f/o gate rows of all weights are pre-halved on the host to
compensate.  Cell state is stored as C = 2c and hidden as H = 2h (weights
consuming h are pre-halved), which lets every elementwise step be a single
fused scalar_tensor_tensor op:

    u     = (Ti + 1) * Tg
    v     = (Tf + 1) * C_prev
    C_new = 0.5*v + u                  (== 2*c_new)
    tanhc = tanh(0.5 * C_new)          (ACT with scale=0.5)
    H_new = (To + 1) * tanhc           (== 2*h_new)

Prediction-phase feedback (prev_y = mean_{t-1}) is folded into the recurrence
as a rank-1 matrix Wfb = Wi0[:,0] (x) (0.5*meanW) applied to H2, so the mean
head never runs inside the loop; means are computed on the host from the
exported H2 states.
"""

import ml_dtypes
import numpy as np

BF16 = ml_dtypes.bfloat16

B = 1024
SEQ, PRED = 192, 96
W = SEQ + PRED  # 288
HID = 128
NCORES = 8
BS = B // NCORES  # 128
IN = 67
KX = IN + 2  # + ones row (bias1) + indicator row (pred feedback bias)
G4 = 4 * HID  # 512
# torch gate order (i, f, g, o) -> device order (i, f, o, g)
GATE_PERM = [0, 1, 3, 2]
X_CHUNK = 16  # scan steps per input-DMA chunk
# packed weight-constant column offsets
WOFF = {"wi0": 0, "wh0": 512, "wi1": 1024, "wh1": 1536, "wfb": 2048,
        "b2m": 2560, "bones": 2688}
WCOLS = 2688 + 512  # 3200


def _perm_rows(w):
    """(4H, X) or (4H,) -> gate-permuted + i/f/o rows halved (tanh trick)."""
    w = w.reshape(4, HID, -1) if w.ndim == 2 else w.reshape(4, HID, 1)
    w = w[GATE_PERM].astype(np.float64).copy()
    w[0] *= 0.5  # i
    w[1] *= 0.5  # f
    w[2] *= 0.5  # o
    return w  # (4, HID, X)


def _as_blocksT(w4):
    """(4, HID, K) -> (K, 4*HID) with gate blocks along columns (lhsT form)."""
    k = w4.shape[2]
    out = np.zeros((k, G4), np.float64)
    for g in range(4):
        out[:, g * HID:(g + 1) * HID] = w4[g].T
    return out


def host_prep(inputs):
    """All data-movement-only preprocessing + weight folding. Returns dict."""
    f32 = np.float32
    ge = np.asarray(inputs["given_enc"], f32)
    x_enc = np.asarray(inputs["x_enc"], f32)
    xm = np.asarray(inputs["x_mark_enc"], f32)
    mx = np.asarray(inputs["meta_x"], f32)
    tembs = [np.asarray(inputs[f"time_emb{i}"], f32) for i in range(3)]
    membs = [np.asarray(inputs[f"meta_emb{i}"], f32) for i in range(2)]

    tcat = ge[:, :, 4:7].astype(np.int32)
    time_feat = np.concatenate(
        [ge[:, :, :4]] + [tembs[i][tcat[:, :, i]] for i in range(3)], axis=-1
    )  # (B, W, 28)
    mcat = mx[:, 2:4].astype(np.int32)
    meta_feat = np.concatenate(
        [mx[:, :2]] + [membs[i][mcat[:, i]] for i in range(2)], axis=-1
    )  # (B, 34)

    nm = x_enc.mean(axis=1, keepdims=True)  # (B,1,1)
    xc = x_enc - nm
    ns = np.sqrt(xc.var(axis=1, keepdims=True) + 1e-5)
    xn = (xc / ns).astype(f32)  # (B, SEQ, 1)

    teacher = np.zeros((B, W, 1), f32)
    teacher[:, 0] = xn[:, 0]
    teacher[:, 1:SEQ] = xn[:, : SEQ - 1]
    ones = np.ones((B, W, 1), f32)
    ind = np.zeros((B, W, 1), f32)
    ind[:, SEQ:] = 1.0
    xfeat = np.concatenate(
        [teacher, time_feat, xm,
         np.broadcast_to(meta_feat[:, None, :], (B, W, 34)), ones, ind],
        axis=-1,
    )  # (B, W, 69)

    Wi0 = np.asarray(inputs["W_ih0"], np.float64)  # (512, 67)
    Wh0 = np.asarray(inputs["W_hh0"], np.float64)
    Wi1 = np.asarray(inputs["W_ih1"], np.float64)
    Wh1 = np.asarray(inputs["W_hh1"], np.float64)
    b1 = np.asarray(inputs["b_ih0"], np.float64) + np.asarray(inputs["b_hh0"], np.float64)
    b2 = np.asarray(inputs["b_ih1"], np.float64) + np.asarray(inputs["b_hh1"], np.float64)
    meanW = np.asarray(inputs["mean_W"], np.float64)  # (1, 128)
    mean_b = float(np.asarray(inputs["mean_b"]).reshape(()))

    wfb_full = Wi0[:, 0:1] @ meanW  # (512, 128), consumes h2
    bias_fb = Wi0[:, 0] * mean_b  # (512,)

    wi0T = _as_blocksT(_perm_rows(Wi0))  # (67, 512)
    wi0T_aug = np.zeros((KX, G4), np.float64)
    wi0T_aug[:IN] = wi0T
    wi0T_aug[IN] = _as_blocksT(_perm_rows(b1)).reshape(G4)  # ones row: bias1
    wi0T_aug[IN + 1] = _as_blocksT(_perm_rows(bias_fb)).reshape(G4)  # indicator
    wh0T = _as_blocksT(_perm_rows(Wh0))  # (128, 512)
    wi1T = _as_blocksT(_perm_rows(Wi1))
    wh1T = _as_blocksT(_perm_rows(Wh1))
    wfbT = _as_blocksT(_perm_rows(wfb_full))  # (128, 512)

    b2m = _perm_rows(b2).reshape(4, HID)
    bones = np.zeros((4, G4), f32)
    for g in range(4):
        bones[g, g * HID:(g + 1) * HID] = 1.0

    # per-core transposed inputs: (KX, W*BS), feature on partitions
    xt_cores = []
    for c in range(NCORES):
        xf = xfeat[c * BS:(c + 1) * BS]  # (BS, W, KX)
        xt = np.ascontiguousarray(xf.transpose(2, 1, 0)).reshape(KX, W * BS)
        xt_cores.append(xt.astype(BF16))

    # Pack every weight into one (128, WCOLS) tensor -> single DMA -> single
    # semaphore for all weight-consuming matmuls (walrus allows only one sync
    # wait per LDWEIGHTS).
    wconst = np.zeros((HID, WCOLS), BF16)
    wconst[:KX, WOFF["wi0"]:WOFF["wi0"] + G4] = wi0T_aug
    wconst[:, WOFF["wh0"]:WOFF["wh0"] + G4] = wh0T
    wconst[:, WOFF["wi1"]:WOFF["wi1"] + G4] = wi1T
    wconst[:, WOFF["wh1"]:WOFF["wh1"] + G4] = wh1T
    wconst[:, WOFF["wfb"]:WOFF["wfb"] + G4] = wfbT
    wconst[:4, WOFF["b2m"]:WOFF["b2m"] + HID] = b2m
    wconst[:4, WOFF["bones"]:WOFF["bones"] + G4] = bones

    return dict(
        xt_cores=xt_cores,
        wconst=wconst,
        weights=dict(
            wi0=wi0T_aug.astype(f32), wh0=wh0T.astype(f32),
            wi1=wi1T.astype(f32), wh1=wh1T.astype(f32),
            wfb=wfbT.astype(f32), b2m=b2m.astype(f32), bones=bones,
        ),
        meanW_h=meanW.astype(f32), mean_b=mean_b,
        norm_std=ns.astype(f32), norm_mean=nm.astype(f32),
    )


def host_post(h2_cores, prep):
    """h2_cores: list of (PRED, HID, BS) arrays of H2=2*h2. -> (B, PRED, 1)."""
    meanW_h = prep["meanW_h"][0]  # (HID,)
    out = np.empty((B, PRED, 1), np.float32)
    for c, h2 in enumerate(h2_cores):
        # mean_norm[t, b] = sum_h meanW_h[h] * H2[t, h, b] + mean_b
        mn = np.einsum("h,thb->bt", meanW_h, h2.astype(np.float32)) + prep["mean_b"]
        out[c * BS:(c + 1) * BS, :, 0] = mn
    out = out * prep["norm_std"] + prep["norm_mean"]
    return out.astype(np.float32)


def build_bass():
    import concourse.bass as bass  # noqa: F401
    import concourse.tile as tile
    from concourse import bacc, mybir

    f32 = mybir.dt.float32
    bf16 = mybir.dt.bfloat16
    AF = mybir.ActivationFunctionType
    ALU = mybir.AluOpType
    OFF = 8  # teacher-phase layer-2 lag (decouples the two recurrence chains)

    nc = bacc.Bacc("TRN2", target_bir_lowering=False, num_devices=NCORES)
    xt_d = nc.dram_tensor("xt", [KX, W * BS], bf16, kind="ExternalInput")
    wc_d = nc.dram_tensor("wconst", [HID, WCOLS], bf16, kind="ExternalInput")
    h2out_d = nc.dram_tensor("h2out", [PRED, HID, BS], bf16, kind="ExternalOutput")

    with tile.TileContext(nc) as tc:
        with (
            tc.tile_pool(name="const", bufs=1) as const,
            tc.tile_pool(name="xin", bufs=3) as xin,
            tc.tile_pool(name="h1p", bufs=OFF + 3) as h1p,
            tc.tile_pool(name="st", bufs=3) as st,
            tc.tile_pool(name="work", bufs=3) as work,
            tc.tile_pool(name="ps", bufs=2, space="PSUM") as ps,
        ):
            wc = const.tile([HID, WCOLS], bf16, tag="wc", name="wc")
            nc.sync.dma_start(out=wc, in_=wc_d[:, :])
            wt = {
                "wi0": wc[:KX, WOFF["wi0"]:WOFF["wi0"] + G4],
                "wh0": wc[:, WOFF["wh0"]:WOFF["wh0"] + G4],
                "wi1": wc[:, WOFF["wi1"]:WOFF["wi1"] + G4],
                "wh1": wc[:, WOFF["wh1"]:WOFF["wh1"] + G4],
                "wfb": wc[:, WOFF["wfb"]:WOFF["wfb"] + G4],
                "b2m": wc[:4, WOFF["b2m"]:WOFF["b2m"] + HID],
                "bones": wc[:4, WOFF["bones"]:WOFF["bones"] + G4],
            }

            def blk(ap, g):
                return ap[:, g * HID:(g + 1) * HID]

            def new_zero(pool, tag, dt):
                t = pool.tile([HID, BS], dt, tag=tag, name=tag)
                nc.vector.memset(t, 0.0)
                return t

            h1 = new_zero(h1p, "h1", bf16)
            h2 = new_zero(st, "h2", bf16)
            c1 = new_zero(st, "c1", f32)
            c2 = new_zero(st, "c2", f32)
            h1_hist = {-1: h1}

            # one cell: psum gates -> (tanh, sigma, u, v, c, tanh_c, h)
            def cell_tanh(g_ps, tag):
                t = work.tile([HID, G4], bf16, tag=f"t{tag}", name=f"t{tag}")
                nc.scalar.activation(out=t, in_=g_ps, func=AF.Tanh)
                return t

            def cell_uvc(t, cprev, tag):
                s = work.tile([HID, 3 * HID], bf16, tag=f"s{tag}", name=f"s{tag}")
                nc.vector.tensor_scalar(out=s, in0=t[:, 0:3 * HID], scalar1=1.0,
                                        scalar2=0.5, op0=ALU.add, op1=ALU.mult)
                u = work.tile([HID, BS], bf16, tag=f"u{tag}", name=f"u{tag}")
                nc.vector.tensor_mul(out=u, in0=s[:, 0:HID], in1=t[:, 3 * HID:G4])
                v = work.tile([HID, BS], f32, tag=f"v{tag}", name=f"v{tag}")
                nc.vector.tensor_mul(out=v, in0=s[:, HID:2 * HID], in1=cprev)
                c_new = st.tile([HID, BS], f32, tag=f"c{tag}n", name=f"c{tag}n")
                nc.vector.tensor_add(out=c_new, in0=u, in1=v)
                return s, c_new

            def cell_tc(c_new, tag):
                tc_ = work.tile([HID, BS], bf16, tag=f"tc{tag}", name=f"tc{tag}")
                nc.scalar.activation(out=tc_, in_=c_new, func=AF.Tanh)
                return tc_

            def cell_h(s, tc_, pool, tag):
                h_new = pool.tile([HID, BS], bf16, tag=f"h{tag}", name=f"h{tag}")
                nc.vector.tensor_mul(out=h_new, in0=s[:, 2 * HID:3 * HID], in1=tc_)
                return h_new

            xt_sb = None

            def xcol_for(t):
                nonlocal xt_sb
                if t % X_CHUNK == 0:
                    nsteps = min(X_CHUNK, W - t)
                    xt_sb = xin.tile([KX, X_CHUNK * BS], bf16, tag="xt",
                                     name="xt_sb")
                    nc.sync.dma_start(out=xt_sb[:, :nsteps * BS],
                                      in_=xt_d[:, t * BS:(t + nsteps) * BS])
                return xt_sb[:, (t % X_CHUNK) * BS:(t % X_CHUNK + 1) * BS]

            # ---------------- teacher phase: L1 stream + L2 stream (lag OFF)
            for i in range(SEQ + OFF):
                j = i - OFF
                g2 = g1 = None
                if 0 <= j:
                    g2 = ps.tile([HID, G4], f32, tag="g2", name="g2")
                    nc.tensor.matmul(g2, lhsT=wt["b2m"], rhs=wt["bones"],
                                     start=True, stop=False)
                    for g in range(4):
                        nc.tensor.matmul(blk(g2, g), lhsT=blk(wt["wi1"], g),
                                         rhs=h1_hist[j], start=False, stop=False)
                    for g in range(4):
                        nc.tensor.matmul(blk(g2, g), lhsT=blk(wt["wh1"], g),
                                         rhs=h2, start=False, stop=(g == 3))
                if i < SEQ:
                    xcol = xcol_for(i)
                    g1 = ps.tile([HID, G4], f32, tag="g1", name="g1")
                    for g in range(4):
                        nc.tensor.matmul(blk(g1, g), lhsT=blk(wt["wi0"], g),
                                         rhs=xcol, start=(g == 0), stop=False)
                    for g in range(4):
                        nc.tensor.matmul(blk(g1, g), lhsT=blk(wt["wh0"], g),
                                         rhs=h1_hist[i - 1], start=False,
                                         stop=(g == 3))
                t2 = cell_tanh(g2, "2") if g2 is not None else None
                t1 = cell_tanh(g1, "1") if g1 is not None else None
                if t2 is not None:
                    s2, c2 = cell_uvc(t2, c2, "2")
                if t1 is not None:
                    s1, c1 = cell_uvc(t1, c1, "1")
                if t2 is not None:
                    tc2 = cell_tc(c2, "2")
                if t1 is not None:
                    tc1 = cell_tc(c1, "1")
                if t2 is not None:
                    h2 = cell_h(s2, tc2, st, "2")
                if t1 is not None:
                    h1_hist[i] = cell_h(s1, tc1, h1p, "1")
                    h1_hist.pop(i - OFF - 1, None)

            # ---------------- prediction phase: serial, hoisted issue order
            h1 = h1_hist[SEQ - 1]
            for t in range(SEQ, W):
                xcol = xcol_for(t)
                g1 = ps.tile([HID, G4], f32, tag="g1", name="g1")
                for g in range(4):
                    nc.tensor.matmul(blk(g1, g), lhsT=blk(wt["wi0"], g),
                                     rhs=xcol, start=(g == 0), stop=False)
                for g in range(4):
                    nc.tensor.matmul(blk(g1, g), lhsT=blk(wt["wh0"], g),
                                     rhs=h1, start=False, stop=False)
                for g in range(4):
                    nc.tensor.matmul(blk(g1, g), lhsT=blk(wt["wfb"], g),
                                     rhs=h2, start=False, stop=(g == 3))
                g2 = ps.tile([HID, G4], f32, tag="g2", name="g2")
                nc.tensor.matmul(g2, lhsT=wt["b2m"], rhs=wt["bones"],
                                 start=True, stop=False)
                for g in range(4):
                    nc.tensor.matmul(blk(g2, g), lhsT=blk(wt["wh1"], g),
                                     rhs=h2, start=False, stop=False)
                t1 = cell_tanh(g1, "1")
                s1, c1 = cell_uvc(t1, c1, "1")
                tc1 = cell_tc(c1, "1")
                h1 = cell_h(s1, tc1, h1p, "1")
                for g in range(4):
                    nc.tensor.matmul(blk(g2, g), lhsT=blk(wt["wi1"], g),
                                     rhs=h1, start=False, stop=(g == 3))
                t2 = cell_tanh(g2, "2")
                s2, c2 = cell_uvc(t2, c2, "2")
                tc2 = cell_tc(c2, "2")
                h2 = cell_h(s2, tc2, st, "2")
                nc.sync.dma_start(out=h2out_d[t - SEQ], in_=h2)
    nc.compile()
    return nc


_BASS_CACHE = {}


def _get_bass():
    if "nc" not in _BASS_CACHE:
        _BASS_CACHE["nc"] = build_bass()
    return _BASS_CACHE["nc"]


def run(inputs, trace=False):
    """Returns (output, BassKernelResults)."""
    from concourse.bass_utils import run_bass_kernel_spmd

    prep = host_prep(inputs)
    nc = _get_bass()
    in_maps = [{"xt": prep["xt_cores"][c], "wconst": prep["wconst"]}
               for c in range(NCORES)]
    res = run_bass_kernel_spmd(nc, in_maps, core_ids=list(range(NCORES)),
                               trace=trace)
    h2_cores = [r["h2out"] for r in res.results]
    return host_post(h2_cores, prep), res


def kernel(**inputs) -> np.ndarray:
    out, _ = run(inputs, trace=False)
    return out
